# revision 8
# baseline (speedup 1.0000x reference)
"""Per-column activation-select kernel for Trainium2 (8 NeuronCores, SPMD).

Problem: out[b, n] = act_{codes[n]}(x[b, n]) with 6 activations
(relu, sigmoid, tanh, elu, leaky_relu(0.01), gelu-tanh-approx),
x: [64, 128, 56, 56] f32, codes: [401408] int32.

Strategy (v3, layout-sorted + dual DMA queues):
  - Shard batch (64) across 8 cores -> 8 rows/core.
  - Host-side layout transform: sort the N=401408 feature columns by
    activation code and pack them into a [128, F2] layout where each
    code owns a contiguous run of f-columns (segments padded to whole
    128-partition columns; F2 <= 3136+6).  Pure data movement, done
    once per input; segment boundaries are baked into the compiled
    module (cached per code histogram).
  - Segment order relu|leaky|elu|sigmoid|tanh|gelu matches execution
    order, so each half of a row leaves for HBM in one contiguous DMA
    as soon as its last segment is written.
  - Work split: DVE does relu (ts_max), leaky (custom PRELU op),
    elu min + fused tail; ACT does exp/sigmoid/tanh/gelu segments,
    all writing in-place into the x tile (no out tiles, no masks,
    no copy_predicated, no gpsimd).
  - Input DMAs post on the ACT HWDGE queue, output DMAs on the sync
    HWDGE queue -> reads and writes stream on independent queues.
  - 4-row chunks, ACT table-set order alternated to minimize loads.
"""
import sys

import numpy as np

sys.path.insert(0, "/opt/trn_rl_repo")

B, C, H, W = 64, 128, 56, 56
N = C * H * W            # 401408
P = 128                  # SBUF partitions
NCORES = 8
RPC = B // NCORES        # rows per core
CHUNK = 2                # rows per function-major chunk
NUM_ACTS = 6
SEG_ORDER = (0, 4, 3, 1, 2, 5)   # relu, leaky, elu | sigmoid, tanh, gelu

_cache = {}


def _register_dve_op(name, spec_body, reference):
    """Register (idempotently) a custom DVE op, mirroring dve_ops bookkeeping."""
    import re

    from concourse.dve_ops import OPS, DveOp
    from concourse.dve_spec import Spec

    for op in OPS:
        if op.name == name:
            return op
    op = DveOp(name, Spec(body=spec_body, reference=reference),
               subdim=False, uops_sha={})
    OPS.append(op)
    from concourse import dve_ops as _do

    _do._SUB_OPCODE_FOR_NAME[op.name] = _do._CUSTOM_DVE_ROW_BASE + len(OPS) - 1
    assert _do._SUB_OPCODE_FOR_NAME[op.name] < 0x20
    _do.CUSTOM_DVE_SPECS[op.name] = op.spec
    for ver in ("v3", "v4"):
        try:
            op.compile(ver)
        except ValueError as e:
            m = re.search(r'\]="([0-9a-f]+)"', str(e))
            op.uops_sha[ver] = m.group(1)
            op.compile(ver)
    return op


def _elu_fuse_op():
    """out = relu(in0) - 1 + in1  (elu when in1 = exp(min(x, 0)))."""
    if "elu_op" not in _cache:
        from concourse.dve_spec import One, Src0, Src1, relu

        _cache["elu_op"] = _register_dve_op(
            "ELU_FUSE_ANT",
            relu(Src0) - One + Src1,
            lambda in0, in1: np.maximum(in0, 0) - 1 + in1,
        )
    return _cache["elu_op"]


def _prelu_op():
    """out = relu(in0) + s0 * min(in0, 0)  (leaky relu, slope s0)."""
    if "prelu_op" not in _cache:
        from concourse.dve_spec import C0, Src0, Zero, minn, relu

        _cache["prelu_op"] = _register_dve_op(
            "PRELU_SLOPE_ANT",
            relu(Src0) + C0 * minn(Src0, Zero),
            lambda in0, in1, s0, s1, imm2:
                np.maximum(in0, 0) + s0 * np.minimum(in0, 0),
        )
    return _cache["prelu_op"]


def _layout(codes: np.ndarray):
    """Sorted-by-code packed layout in SEG_ORDER.

    Returns (idx, valid, segs, F2): idx[p, f] = source column for slot
    (p, f) (0 for pad), valid[p, f] = not-pad, segs = tuple of
    (code, f_start, f_end) in SEG_ORDER, F2 = total f-columns.
    """
    order = np.argsort(codes, kind="stable")
    cnt = np.bincount(codes, minlength=NUM_ACTS)
    starts = np.concatenate([[0], np.cumsum(cnt)])
    f2 = int(sum(-(-int(c) // P) for c in cnt))
    idx = np.zeros((P, f2), dtype=np.int64)
    valid = np.zeros((P, f2), dtype=bool)
    segs = []
    f0 = 0
    for k in SEG_ORDER:
        c = int(cnt[k])
        if c == 0:
            continue
        ext = -(-c // P)
        cols = order[starts[k]:starts[k] + c]
        j = np.arange(c)
        idx[j % P, f0 + j // P] = cols
        valid[j % P, f0 + j // P] = True
        segs.append((k, f0, f0 + ext))
        f0 += ext
    return idx, valid, tuple(segs), f2


def _layout_cached(codes: np.ndarray):
    key = codes.tobytes()
    ent = _cache.get("layout")
    if ent is None or ent[0] != key:
        ent = (key, _layout(codes))
        _cache["layout"] = ent
    return ent[1]


def _build_module(segs: tuple, F2: int):
    import concourse.bacc as bacc
    import concourse.mybir as mybir
    from concourse import tile

    AF = mybir.ActivationFunctionType
    FP32 = mybir.dt.float32

    AFM = {1: AF.Sigmoid, 2: AF.Tanh, 5: AF.Gelu_apprx_tanh}
    seg = {k: (a, b) for k, a, b in segs}

    nc = bacc.Bacc(target_bir_lowering=False, debug=False)
    x_in = nc.dram_tensor("x", [RPC, P, F2], FP32, kind="ExternalInput").ap()
    out = nc.dram_tensor("out", [RPC, P, F2], FP32, kind="ExternalOutput").ap()

    with tile.TileContext(nc) as tc:
        with (
            tc.tile_pool(name="xp", bufs=RPC) as xpool,
            tc.tile_pool(name="sm", bufs=CHUNK) as small,
        ):
            xt = [None] * RPC
            # all input DMAs up-front on the ACT HWDGE queue
            for r in range(RPC):
                xt[r] = xpool.tile([P, F2], FP32, tag="x", name=f"xt{r}")
                nc.scalar.dma_start(xt[r][:], x_in[r])

            def dve_half(rows):
                # relu + leaky + elu-min on DVE, in-place
                mn = {}
                if 0 in seg:
                    a, b = seg[0]
                    for r in rows:
                        nc.vector.tensor_scalar_max(
                            xt[r][:, a:b], xt[r][:, a:b], 0.0
                        )
                if 4 in seg:
                    a, b = seg[4]
                    for r in rows:
                        nc.vector._custom_dve(
                            _prelu_op(), out=xt[r][:, a:b],
                            in0=xt[r][:, a:b], s0=0.01,
                        )
                if 3 in seg:
                    a, b = seg[3]
                    for r in rows:
                        mn[r] = small.tile(
                            [P, b - a], FP32, tag="mn", name=f"mn{r}"
                        )
                        nc.vector.tensor_scalar_min(
                            mn[r][:], xt[r][:, a:b], 0.0
                        )
                return mn

            def elu_tail(rows, mn):
                # exp on ACT, fused elu tail on DVE
                if 3 in seg:
                    a, b = seg[3]
                    e = {}
                    for r in rows:
                        e[r] = small.tile(
                            [P, b - a], FP32, tag="e", name=f"e{r}"
                        )
                        nc.scalar.activation(e[r][:], mn[r][:], AF.Exp)
                    for r in rows:
                        nc.vector._custom_dve(
                            _elu_fuse_op(), out=xt[r][:, a:b],
                            in0=xt[r][:, a:b], in1=e[r][:],
                        )

            def act_half(rows, klist):
                # sigmoid/tanh/gelu segments on ACT, in-place
                for k in klist:
                    if k not in seg:
                        continue
                    a, b = seg[k]
                    for r in rows:
                        nc.scalar.activation(
                            xt[r][:, a:b], xt[r][:, a:b], AFM[k]
                        )

            def row_out(rows):
                # one full-row out DMA as soon as the row is complete
                for r in rows:
                    nc.sync.dma_start(out[r], xt[r][:])

            nchunks = -(-RPC // CHUNK)
            for ci in range(nchunks):
                rows = list(range(ci * CHUNK, min((ci + 1) * CHUNK, RPC)))
                if ci % 2 == 0:
                    # ACT sets: S(sig,tanh) -> G -> E; DVE runs relu/
                    # leaky/min concurrently with sig/tanh/gelu
                    mn = dve_half(rows)
                    act_half(rows, [1, 2, 5])
                    elu_tail(rows, mn)
                else:
                    # reverse set order: E -> G -> S (exp still loaded)
                    mn = dve_half(rows)
                    elu_tail(rows, mn)
                    act_half(rows, [5, 2, 1])
                row_out(rows)

    nc.compile()
    return nc


def _get_module(segs: tuple, F2: int):
    key = ("nc", segs, F2)
    if key not in _cache:
        _cache[key] = _build_module(segs, F2)
    return _cache[key]


def _prepare(x: np.ndarray, act_codes: np.ndarray):
    """Shared host-side prep: layout, module, per-core input maps."""
    x = np.ascontiguousarray(np.asarray(x, dtype=np.float32))
    codes = np.asarray(act_codes, dtype=np.int32).ravel()
    idx, valid, segs, F2 = _layout_cached(codes)
    nc = _get_module(segs, F2)
    xp = x.reshape(B, N)[:, idx]               # [B, P, F2] packed layout
    in_maps = [{"x": xp[c * RPC:(c + 1) * RPC]} for c in range(NCORES)]
    return nc, in_maps, idx, valid


def kernel(x: np.ndarray, act_codes: np.ndarray) -> np.ndarray:
    from concourse.bass_utils import run_bass_kernel_spmd

    nc, in_maps, idx, valid = _prepare(x, act_codes)
    res = run_bass_kernel_spmd(nc, in_maps, list(range(NCORES)))
    packed = np.concatenate(
        [res.results[c]["out"] for c in range(NCORES)], axis=0
    )                                          # [B, P, F2]
    outf = np.empty((B, N), dtype=np.float32)
    outf[:, idx[valid]] = packed[:, valid]
    return outf.reshape(B, C, H, W)


# revision 9
# speedup vs baseline: 1.1474x; 1.1474x over previous
"""Per-column activation-select kernel for Trainium2 (8 NeuronCores, SPMD).

Problem: out[b, n] = act_{codes[n]}(x[b, n]) with 6 activations
(relu, sigmoid, tanh, elu, leaky_relu(0.01), gelu-tanh-approx),
x: [64, 128, 56, 56] f32, codes: [401408] int32.

Strategy (v3, layout-sorted + dual DMA queues):
  - Shard batch (64) across 8 cores -> 8 rows/core.
  - Host-side layout transform: sort the N=401408 feature columns by
    activation code and pack them into a [128, F2] layout where each
    code owns a contiguous run of f-columns (segments padded to whole
    128-partition columns; F2 <= 3136+6).  Pure data movement, done
    once per input; segment boundaries are baked into the compiled
    module (cached per code histogram).
  - Segment order relu|leaky|elu|sigmoid|tanh|gelu matches execution
    order, so each half of a row leaves for HBM in one contiguous DMA
    as soon as its last segment is written.
  - Work split: DVE does relu (ts_max), leaky (custom PRELU op),
    elu min + fused tail; ACT does exp/sigmoid/tanh/gelu segments,
    all writing in-place into the x tile (no out tiles, no masks,
    no copy_predicated, no gpsimd).
  - Input DMAs post on the ACT HWDGE queue, output DMAs on the sync
    HWDGE queue -> reads and writes stream on independent queues.
  - 4-row chunks, ACT table-set order alternated to minimize loads.
"""
import sys

import numpy as np

sys.path.insert(0, "/opt/trn_rl_repo")

B, C, H, W = 64, 128, 56, 56
N = C * H * W            # 401408
P = 128                  # SBUF partitions
NCORES = 8
RPC = B // NCORES        # rows per core
CHUNK = 2                # rows per function-major chunk
NUM_ACTS = 6
SEG_ORDER = (0, 4, 3, 1, 2, 5)   # relu, leaky, elu | sigmoid, tanh, gelu

_cache = {}


def _register_dve_op(name, spec_body, reference):
    """Register (idempotently) a custom DVE op, mirroring dve_ops bookkeeping."""
    import re

    from concourse.dve_ops import OPS, DveOp
    from concourse.dve_spec import Spec

    for op in OPS:
        if op.name == name:
            return op
    op = DveOp(name, Spec(body=spec_body, reference=reference),
               subdim=False, uops_sha={})
    OPS.append(op)
    from concourse import dve_ops as _do

    _do._SUB_OPCODE_FOR_NAME[op.name] = _do._CUSTOM_DVE_ROW_BASE + len(OPS) - 1
    assert _do._SUB_OPCODE_FOR_NAME[op.name] < 0x20
    _do.CUSTOM_DVE_SPECS[op.name] = op.spec
    for ver in ("v3", "v4"):
        try:
            op.compile(ver)
        except ValueError as e:
            m = re.search(r'\]="([0-9a-f]+)"', str(e))
            op.uops_sha[ver] = m.group(1)
            op.compile(ver)
    return op


def _elu_fuse_op():
    """out = relu(in0) - 1 + in1  (elu when in1 = exp(min(x, 0)))."""
    if "elu_op" not in _cache:
        from concourse.dve_spec import One, Src0, Src1, relu

        _cache["elu_op"] = _register_dve_op(
            "ELU_FUSE_ANT",
            relu(Src0) - One + Src1,
            lambda in0, in1: np.maximum(in0, 0) - 1 + in1,
        )
    return _cache["elu_op"]


def _prelu_op():
    """out = relu(in0) + s0 * min(in0, 0)  (leaky relu, slope s0)."""
    if "prelu_op" not in _cache:
        from concourse.dve_spec import C0, Src0, Zero, minn, relu

        _cache["prelu_op"] = _register_dve_op(
            "PRELU_SLOPE_ANT",
            relu(Src0) + C0 * minn(Src0, Zero),
            lambda in0, in1, s0, s1, imm2:
                np.maximum(in0, 0) + s0 * np.minimum(in0, 0),
        )
    return _cache["prelu_op"]


def _layout(codes: np.ndarray):
    """Sorted-by-code packed layout in SEG_ORDER.

    Returns (idx, valid, segs, F2): idx[p, f] = source column for slot
    (p, f) (0 for pad), valid[p, f] = not-pad, segs = tuple of
    (code, f_start, f_end) in SEG_ORDER, F2 = total f-columns.
    """
    order = np.argsort(codes, kind="stable")
    cnt = np.bincount(codes, minlength=NUM_ACTS)
    starts = np.concatenate([[0], np.cumsum(cnt)])
    f2 = int(sum(-(-int(c) // P) for c in cnt))
    idx = np.zeros((P, f2), dtype=np.int64)
    valid = np.zeros((P, f2), dtype=bool)
    segs = []
    f0 = 0
    for k in SEG_ORDER:
        c = int(cnt[k])
        if c == 0:
            continue
        ext = -(-c // P)
        cols = order[starts[k]:starts[k] + c]
        j = np.arange(c)
        idx[j % P, f0 + j // P] = cols
        valid[j % P, f0 + j // P] = True
        segs.append((k, f0, f0 + ext))
        f0 += ext
    return idx, valid, tuple(segs), f2


def _layout_cached(codes: np.ndarray):
    key = codes.tobytes()
    ent = _cache.get("layout")
    if ent is None or ent[0] != key:
        ent = (key, _layout(codes))
        _cache["layout"] = ent
    return ent[1]


def _build_module(segs: tuple, F2: int):
    import concourse.bacc as bacc
    import concourse.mybir as mybir
    from concourse import tile

    AF = mybir.ActivationFunctionType
    FP32 = mybir.dt.float32

    AFM = {1: AF.Sigmoid, 2: AF.Tanh, 5: AF.Gelu_apprx_tanh}
    seg = {k: (a, b) for k, a, b in segs}

    nc = bacc.Bacc(target_bir_lowering=False, debug=False)
    x_in = nc.dram_tensor("x", [RPC, P, F2], FP32, kind="ExternalInput").ap()
    out = nc.dram_tensor("out", [RPC, P, F2], FP32, kind="ExternalOutput").ap()

    with tile.TileContext(nc) as tc:
        with (
            tc.tile_pool(name="xp", bufs=RPC) as xpool,
            tc.tile_pool(name="sm", bufs=CHUNK) as small,
        ):
            xt = [None] * RPC
            # all input DMAs up-front on the ACT HWDGE queue
            for r in range(RPC):
                xt[r] = xpool.tile([P, F2], FP32, tag="x", name=f"xt{r}")
                nc.scalar.dma_start(xt[r][:], x_in[r])

            def dve_half(rows):
                # relu + leaky + elu-min on DVE, in-place
                mn = {}
                if 0 in seg:
                    a, b = seg[0]
                    for r in rows:
                        nc.vector.tensor_scalar_max(
                            xt[r][:, a:b], xt[r][:, a:b], 0.0
                        )
                if 4 in seg:
                    a, b = seg[4]
                    for r in rows:
                        nc.vector._custom_dve(
                            _prelu_op(), out=xt[r][:, a:b],
                            in0=xt[r][:, a:b], s0=0.01,
                        )
                if 3 in seg:
                    a, b = seg[3]
                    for r in rows:
                        mn[r] = small.tile(
                            [P, b - a], FP32, tag="mn", name=f"mn{r}"
                        )
                        nc.vector.tensor_scalar_min(
                            mn[r][:], xt[r][:, a:b], 0.0
                        )
                return mn

            def elu_tail(rows, mn):
                # exp on ACT, fused elu tail on DVE
                if 3 in seg:
                    a, b = seg[3]
                    e = {}
                    for r in rows:
                        e[r] = small.tile(
                            [P, b - a], FP32, tag="e", name=f"e{r}"
                        )
                        nc.scalar.activation(e[r][:], mn[r][:], AF.Exp)
                    for r in rows:
                        nc.vector._custom_dve(
                            _elu_fuse_op(), out=xt[r][:, a:b],
                            in0=xt[r][:, a:b], in1=e[r][:],
                        )

            def act_half(rows, klist):
                # sigmoid/tanh/gelu segments on ACT, in-place
                for k in klist:
                    if k not in seg:
                        continue
                    a, b = seg[k]
                    for r in rows:
                        nc.scalar.activation(
                            xt[r][:, a:b], xt[r][:, a:b], AFM[k]
                        )

            def row_out(rows):
                # one full-row out DMA as soon as the row is complete
                for r in rows:
                    nc.sync.dma_start(out[r], xt[r][:])

            # exp/fuse go EARLY in each chunk: the Tile scheduler's
            # counting semaphores are position-based, so a late fuse
            # makes every out-DMA transitively wait on later rows'
            # hoisted DVE work (measured: outs stalled to t=57us).
            # Each row's last writer is an ACT op (gelu / sigmoid).
            nchunks = -(-RPC // CHUNK)
            for ci in range(nchunks):
                rows = list(range(ci * CHUNK, min((ci + 1) * CHUNK, RPC)))
                if ci % 2 == 0:
                    # ACT sets: E -> S(sig,tanh) -> G
                    mn = dve_half(rows)
                    elu_tail(rows, mn)
                    act_half(rows, [1, 2, 5])
                else:
                    # G -> E -> S: reuses G table from previous chunk
                    mn = dve_half(rows)
                    act_half(rows, [5])
                    elu_tail(rows, mn)
                    act_half(rows, [2, 1])
                row_out(rows)

    nc.compile()
    return nc


def _get_module(segs: tuple, F2: int):
    key = ("nc", segs, F2)
    if key not in _cache:
        _cache[key] = _build_module(segs, F2)
    return _cache[key]


def _prepare(x: np.ndarray, act_codes: np.ndarray):
    """Shared host-side prep: layout, module, per-core input maps."""
    x = np.ascontiguousarray(np.asarray(x, dtype=np.float32))
    codes = np.asarray(act_codes, dtype=np.int32).ravel()
    idx, valid, segs, F2 = _layout_cached(codes)
    nc = _get_module(segs, F2)
    xp = x.reshape(B, N)[:, idx]               # [B, P, F2] packed layout
    in_maps = [{"x": xp[c * RPC:(c + 1) * RPC]} for c in range(NCORES)]
    return nc, in_maps, idx, valid


def kernel(x: np.ndarray, act_codes: np.ndarray) -> np.ndarray:
    from concourse.bass_utils import run_bass_kernel_spmd

    nc, in_maps, idx, valid = _prepare(x, act_codes)
    res = run_bass_kernel_spmd(nc, in_maps, list(range(NCORES)))
    packed = np.concatenate(
        [res.results[c]["out"] for c in range(NCORES)], axis=0
    )                                          # [B, P, F2]
    outf = np.empty((B, N), dtype=np.float32)
    outf[:, idx[valid]] = packed[:, valid]
    return outf.reshape(B, C, H, W)
